# revision 20
# baseline (speedup 1.0000x reference)
"""Trainium2 Bass kernel for nn_CRF (torchcrf-style loss + viterbi decode).

Full inputs in, full outputs out. Data-parallel over 8 NeuronCores:
core c handles sequences [c*512, (c+1)*512), laid out as 4 groups x 128
partitions (seq s = c*512 + g*128 + p).

Per-core algorithm (validated vs reference in numpy mirror):
- Viterbi forward (max-plus) on DVE over augmented state rows (10 tags +
  sigma sink per group). Masking is folded into emissions: tag rows get
  -1e4 per padded step (decays frozen seqs), sigma row gets -1e30 while
  t < len and 0 after, so the sink captures argmax(score+end) exactly at
  t = len with zero per-step masking ops. Backpointers (global row idx,
  bf16) via is_lt + scalar_tensor_tensor + segmented reduce_min
  (first-index tie-break = jnp.argmax semantics).
- Backtrace: arithmetic one-hot selection on DVE (3 small ops/step) --
  the hw has no per-partition gather.
- Partition function in exp space on PE/ACT: alpha rows [44] x seqs[128];
  per step: PE transpose of the emission slice -> ACT exp -> PE matmul by
  exp(trans) block-diag (with sigma capture rows and per-group rowsum
  rows) -> one DVE mult. exp(em-1e4*pad) == 0 exactly, so padded steps
  zero the tag rows and freeze the sigma row. Rescale every 16 steps by
  approximate 1/rowsum; factors are Ln'd in bulk at the end, so the
  approximation cancels exactly.
- Numerator = logZ of the one-hot-masked model: a second alpha pipeline
  over em'' = em' + (onehot(tag)-1)*1e4. Only the gold path survives
  (exp of others is exactly 0), so its sigma sink = exp(gold score).
- loss partial per core; host sums partials and negates.
"""
import sys

for _p in ("/opt/trn_rl_repo",):
    if _p not in sys.path:
        sys.path.insert(0, _p)

import numpy as np

B, L, T = 4096, 512, 10
R = 11          # rows per group: 10 tags + sigma
G = 4           # seq groups per core
P = 128         # partitions
NCORE = 8
SC = G * P      # seqs per core = 512
TC = 64         # emission chunk length (steps)
NCH = L // TC
RS = 16         # rescale interval
NY = L // RS - 1  # rescale events at t=16,32,...,496
LARGE = 64.0    # bp combine offset

_prog_cache = {}


def _host_consts(start, end, trans):
    """All replicated constant tiles, keyed by dram tensor name."""
    f32 = np.float32
    transP = np.full((R, R), -1e30, f32)          # [j, k]
    transP[:T, :T] = trans.T
    transP[T, :T] = end
    transP[T, T] = 0.0
    startP = np.concatenate([start, [f32(-1e30)]]).astype(f32)   # [R]

    E = np.zeros((G * R, 64 + G), f32)             # lhsT [44, 68]
    for c in range(G):
        for kk in range(T):
            for j in range(T):
                E[c * R + kk, c * R + j] = np.exp(trans[kk, j], dtype=f32)
            E[c * R + kk, c * R + T] = np.exp(end[kk], dtype=f32)
        E[c * R + T, c * R + T] = 1.0
        E[c * R: c * R + R, 64 + c] = 1.0          # rowsum rows (incl sigma)

    idx = np.zeros((G, R), f32)
    for g in range(G):
        idx[g, :] = g * R + np.arange(R)

    rep = np.zeros((G, G * R), f32)
    for c in range(G):
        rep[c, c * R: (c + 1) * R] = 1.0

    gvirt = np.zeros((G * R, P), f32)
    gvirt[T::R, :] = 1.0                           # sigma rows = 1

    startExp = np.zeros((G * R, 1), f32)
    for c in range(G):
        startExp[c * R: c * R + T, 0] = np.exp(start, dtype=f32)

    iotak2 = np.arange(R).astype(f32)
    iotak2[T] = 9.0                                # sigma matches PAD tag

    sigoh = np.zeros((G, R), f32)
    sigoh[:, T] = 1.0

    c = {
        "c_transP": np.tile(transP.reshape(1, R * R), (P, 1)),
        "c_idx": np.tile(idx.reshape(1, G * R), (P, 1)),
        "c_startP": np.tile(np.tile(startP, G).reshape(1, G * R), (P, 1)),
        "c_emv": np.tile(
            np.tile(np.array([-1e4] * T + [0.0], f32), G).reshape(1, G * R),
            (P, 1)),
        "c_iotak2": np.tile(iotak2.reshape(1, R), (P, 1)),
        "c_sigoh": np.tile(sigoh.reshape(1, G * R), (P, 1)).astype(f32),
        "c_Eaug": E,
        "c_ident": np.eye(P, dtype=f32),
        "c_gvirt": gvirt,
        "c_startExp": startExp,
        "c_rep": rep,
        "c_g11": np.tile((np.arange(G, dtype=f32) * R).reshape(1, G), (P, 1)),
        "c_idxfull": np.tile(
            np.broadcast_to(idx[:, None, :], (G, R, R)).reshape(1, G * R * R),
            (P, 1)),
    }
    return {k: np.ascontiguousarray(v) for k, v in c.items()}


def _build(nc, tile_mod, mybir):
    from concourse import bass_isa
    f32, i32 = mybir.dt.float32, mybir.dt.int32
    bf16 = mybir.dt.bfloat16
    Alu = mybir.AluOpType
    Act = mybir.ActivationFunctionType
    AX = mybir.AxisListType.X

    fc = nc.dram_tensor("fc", [SC, L, T], f32, kind="ExternalInput")
    tg = nc.dram_tensor("tg", [SC, L], i32, kind="ExternalInput")
    cshapes = {
        "c_transP": ([P, R * R], f32), "c_idx": ([P, G * R], f32),
        "c_startP": ([P, G * R], f32), "c_emv": ([P, G * R], f32),
        "c_iotak2": ([P, R], f32), "c_sigoh": ([P, G * R], f32),
        "c_Eaug": ([G * R, 64 + G], f32),
        "c_ident": ([P, P], f32), "c_gvirt": ([G * R, P], f32),
        "c_startExp": ([G * R, 1], f32), "c_rep": ([G, G * R], f32),
        "c_g11": ([P, G], f32),
        "c_idxfull": ([P, G * R * R], f32),
    }
    cdram = {k: nc.dram_tensor(k, s, d, kind="ExternalInput")
             for k, (s, d) in cshapes.items()}
    crf = nc.dram_tensor("crf", [SC, L], i32, kind="ExternalOutput")
    loss = nc.dram_tensor("loss", [1, 1], f32, kind="ExternalOutput")

    with tile_mod.TileContext(nc) as tc:
        with tc.tile_pool(name="pers", bufs=1) as pers, \
             tc.tile_pool(name="scr", bufs=2) as scr, \
             tc.tile_pool(name="st", bufs=3) as st, \
             tc.tile_pool(name="fin", bufs=1) as fin, \
             tc.tile_pool(name="ps", bufs=1, space="PSUM") as ps:

            # ---- persistent tiles ----
            em = [pers.tile([P, G * TC * R], f32, tag=f"em{b}", name=f"em{b}")
                  for b in (0, 1)]
            em2 = [pers.tile([P, G * TC * R], f32, tag=f"em2{b}",
                             name=f"em2{b}") for b in (0, 1)]
            bp = pers.tile([P, L * G * R], bf16)
            path = pers.tile([P, L * G], f32)
            tags_f = pers.tile([P, G * L], f32)
            msk = pers.tile([P, G * L], f32)
            ysall = pers.tile([G, max(NY, 1) * P], f32)
            sbc = {k: pers.tile(s, d, tag=k, name=k)
                   for k, (s, d) in cshapes.items()}

            for k in cdram:
                nc.sync.dma_start(sbc[k][:], cdram[k][:])

            # ---- tags + mask ----
            tags_i = fin.tile([P, G * L], i32, tag="i32buf")
            for g in range(G):
                for h in range(2):
                    nc.sync.dma_start(
                        tags_i[h * 64:(h + 1) * 64, g * L:(g + 1) * L],
                        tg[g * P + h * 64:g * P + (h + 1) * 64, :])
            nc.vector.tensor_copy(tags_f[:], tags_i[:])
            nc.vector.tensor_scalar(msk[:], tags_f[:], 9.0, None, Alu.is_lt)

            def load_chunk(ch, buf):
                t0 = ch * TC
                emv = em[buf][:].rearrange("p (t g k) -> p t g k", t=TC, g=G)
                for g in range(G):
                    nc.sync.dma_start(
                        emv[:, :, g, 0:T],
                        fc[g * P:(g + 1) * P, t0:t0 + TC, :])
                mskc = (msk[:].rearrange("p (g t) -> p t g", g=G)
                        [:, t0:t0 + TC, :])
                mneg = scr.tile([P, TC * G], f32, tag="mneg")
                mnegv = mneg[:].rearrange("p (t g) -> p t g", t=TC)
                # (m-1)*1e4 in {0, -1e4}
                nc.vector.tensor_scalar(mnegv, mskc, 1.0, 1e4,
                                        Alu.subtract, op1=Alu.mult)
                nc.vector.tensor_tensor(
                    emv[:, :, :, 0:T], emv[:, :, :, 0:T],
                    mnegv.unsqueeze(3).broadcast_to((P, TC, G, T)), Alu.add)
                # sigma channel: m*(-1e30)
                nc.vector.tensor_scalar(emv[:, :, :, T], mskc, -1e30, None,
                                        Alu.mult)
                # ---- numerator emissions: em'' = em' + (onehot - 1)*1e4
                e2v = em2[buf][:].rearrange("p (t g k) -> p t g k", t=TC, g=G)
                tgc = (tags_f[:].rearrange("p (g t) -> p t g", g=G)
                       [:, t0:t0 + TC, :])
                nc.vector.tensor_tensor(
                    e2v,
                    sbc["c_iotak2"][:].unsqueeze(1).unsqueeze(2)
                    .broadcast_to((P, TC, G, R)),
                    tgc.unsqueeze(3).broadcast_to((P, TC, G, R)),
                    Alu.is_equal)
                nc.vector.tensor_scalar(em2[buf][:], em2[buf][:], 1.0, 1e4,
                                        Alu.subtract, op1=Alu.mult)
                nc.vector.tensor_tensor(em2[buf][:], em2[buf][:], em[buf][:],
                                        Alu.add)

            load_chunk(0, 0)
            load_chunk(1, 1)

            # ---- broadcast helper views ----
            tP = (sbc["c_transP"][:].rearrange("p (j k) -> p j k", j=R)
                  .unsqueeze(1).broadcast_to((P, G, R, R)))
            cidx = (sbc["c_idx"][:].rearrange("p (g k) -> p g k", g=G)
                    .unsqueeze(2).broadcast_to((P, G, R, R)))

            # ---- init (t = 0) ----
            se_t = st.tile([P, G * R], f32, tag="se")
            nc.vector.tensor_tensor(
                se_t[:], sbc["c_startP"][:], em[0][:, 0:G * R], Alu.add)

            def alpha_init(emslice, tag):
                gp = ps.tile([G * R, P], f32, tag="gps", bufs=3, name="gps")
                nc.tensor.transpose(gp[:], emslice, sbc["c_ident"][:])
                gs = st.tile([G * R, P], f32, tag="gsb", name="gsb")
                nc.scalar.activation(gs[:], gp[:], Act.Exp)
                al = st.tile([G * R, P], f32, tag=tag, name=tag)
                nc.vector.tensor_scalar(al[:], gs[:],
                                        sbc["c_startExp"][:, 0:1],
                                        None, Alu.mult)
                return al

            al_t = alpha_init(em[0][:, 0:G * R], "al")
            se2_t = st.tile([P, G * R], f32, tag="se2", name="se2")
            nc.vector.tensor_tensor(
                se2_t[:], sbc["c_startP"][:], em2[0][:, 0:G * R], Alu.add)

            # ---- main loop ----
            for t in range(1, L + 1):
                virtual = (t == L)
                if not virtual:
                    ch, t_ = t // TC, t % TC
                    buf = ch % 2
                    if t_ == 1 and 2 <= ch + 1 < NCH:
                        load_chunk(ch + 1, (ch + 1) % 2)
                    emt = (em[buf][:, t_ * G * R:(t_ + 1) * G * R]
                           .rearrange("p (g k) -> p g k", g=G))
                    em2t = (em2[buf][:, t_ * G * R:(t_ + 1) * G * R]
                            .rearrange("p (g k) -> p g k", g=G))
                else:
                    emt = sbc["c_emv"][:].rearrange("p (g k) -> p g k", g=G)
                    em2t = None

                # ---- viterbi ----
                sev = se_t[:].rearrange("p (g k) -> p g k", g=G)
                cand = scr.tile([P, G * R * R], f32, tag="cand")
                cd = cand[:].rearrange("p (g j k) -> p g j k", g=G, j=R)
                nc.vector.tensor_tensor(
                    cd, sev.unsqueeze(2).broadcast_to((P, G, R, R)), tP,
                    Alu.add)
                seraw = st.tile([P, G * R], f32, tag="seraw")
                srv = seraw[:].rearrange("p (g j) -> p g j", g=G)
                nc.vector.tensor_reduce(srv, cd, AX, Alu.max)
                if not virtual:
                    se_n = st.tile([P, G * R], f32, tag="se", name="se")
                    nc.vector.tensor_tensor(
                        se_n[:].rearrange("p (g k) -> p g k", g=G), srv, emt,
                        Alu.add)
                    se_t = se_n
                neq = scr.tile([P, G * R * R], f32, tag="neq")
                nq = neq[:].rearrange("p (g j k) -> p g j k", g=G, j=R)
                nc.vector.tensor_tensor(
                    nq, cd, srv.unsqueeze(3).broadcast_to((P, G, R, R)),
                    Alu.is_lt)
                bpc = scr.tile([P, G * R * R], f32, tag="bpc")
                bv = bpc[:].rearrange("p (g j k) -> p g j k", g=G, j=R)
                nc.vector.scalar_tensor_tensor(bpc[:], neq[:], LARGE,
                                               sbc["c_idxfull"][:],
                                               Alu.mult, Alu.add)
                nc.vector.tensor_reduce(
                    bp[:, (t - 1) * G * R: t * G * R].rearrange(
                        "p (g j) -> p g j", g=G), bv, AX, Alu.min)

                # ---- two alpha pipelines ----
                def alpha_step(al_in, emflat, tag, yrow):
                    aps = ps.tile([64 + G, P], f32, tag="aps", bufs=3,
                                  name="aps")
                    nc.tensor.matmul(aps[:], sbc["c_Eaug"][:], al_in[:],
                                     start=True, stop=True)
                    if virtual:
                        gs = sbc["c_gvirt"]
                    else:
                        gp = ps.tile([G * R, P], f32, tag="gps", bufs=3,
                                     name="gps")
                        nc.tensor.transpose(gp[:], emflat,
                                            sbc["c_ident"][:])
                        gs = st.tile([G * R, P], f32, tag="gsb", name="gsb")
                        nc.scalar.activation(gs[:], gp[:], Act.Exp)
                    al_n = st.tile([G * R, P], f32, tag=tag, name=tag)
                    nc.vector.tensor_tensor(al_n[:], aps[0:G * R, :], gs[:],
                                            Alu.mult)
                    if not virtual and t % RS == 0:
                        ri = t // RS - 1
                        ysl = ysall[0:G, ri * P:(ri + 1) * P]
                        rr = st.tile([G, P], f32, tag="rr", name="rr")
                        nc.vector.tensor_scalar(rr[:], aps[64:64 + G, :],
                                                1e-12, None, Alu.max)
                        nc.vector.reciprocal(ysl, rr[:])
                        yrep = ps.tile([G * R, P], f32, tag="yrep", bufs=1,
                                       name="yrep")
                        nc.tensor.matmul(yrep[:], sbc["c_rep"][:], ysl,
                                         start=True, stop=True)
                        al_s = st.tile([G * R, P], f32, tag=tag, name=tag)
                        nc.vector.tensor_tensor(al_s[:], al_n[:], yrep[:],
                                                Alu.mult)
                        al_n = al_s
                    return al_n

                al_t = alpha_step(
                    al_t,
                    None if virtual else
                    em[buf][:, t_ * G * R:(t_ + 1) * G * R], "al", 0)
                # numerator recursion (max-plus on one-hot-masked emissions)
                se2v = se2_t[:].rearrange("p (g k) -> p g k", g=G)
                cand2 = scr.tile([P, G * R * R], f32, tag="cand2",
                                 name="cand2")
                cd2 = cand2[:].rearrange("p (g j k) -> p g j k", g=G, j=R)
                nc.vector.tensor_tensor(
                    cd2, se2v.unsqueeze(2).broadcast_to((P, G, R, R)), tP,
                    Alu.add)
                se2_n = st.tile([P, G * R], f32, tag="se2", name="se2")
                s2v = se2_n[:].rearrange("p (g j) -> p g j", g=G)
                nc.vector.tensor_reduce(s2v, cd2, AX, Alu.max)
                if not virtual:
                    se2_m = st.tile([P, G * R], f32, tag="se2", name="se2")
                    nc.vector.tensor_tensor(
                        se2_m[:].rearrange("p (g k) -> p g k", g=G), s2v,
                        em2t, Alu.add)
                    se2_t = se2_m
                else:
                    emvv = sbc["c_emv"][:].rearrange("p (g k) -> p g k", g=G)
                    se2_m = st.tile([P, G * R], f32, tag="se2", name="se2")
                    nc.vector.tensor_tensor(
                        se2_m[:].rearrange("p (g k) -> p g k", g=G), s2v,
                        emvv, Alu.add)
                    se2_t = se2_m

            # ---- logZ + numerator readout (transposed layout) ----
            def sigma_ln(al, tag):
                alT = ps.tile([P, G * R], f32, tag="fint", bufs=1, name=tag)
                nc.tensor.transpose(alT[:], al[:],
                                    sbc["c_ident"][0:G * R, 0:G * R])
                lnS = fin.tile([P, G], f32, tag=tag, name=tag)
                nc.scalar.activation(
                    lnS[:],
                    alT[:].rearrange("p (g k) -> p g k", g=G)[:, :, T],
                    Act.Ln)
                return lnS

            lnS1 = sigma_ln(al_t, "lnS1")
            nc.scalar.activation(ysall[:, 0:NY * P],
                                 ysall[:, 0:NY * P], Act.Ln)
            lnYs = fin.tile([G, P], f32, tag="lnYs")
            nc.vector.tensor_reduce(
                lnYs[:],
                ysall[:, 0:NY * P].rearrange("c (y s) -> c s y", y=NY),
                AX, Alu.add)
            zpsA = ps.tile([P, G], f32, tag="fint", bufs=1, name="zpsA")
            nc.tensor.transpose(zpsA[:], lnYs[:],
                                sbc["c_ident"][0:G, 0:G])
            # num - logZ = SE2[sigma] - (lnS1 - sum(ln y))
            diff = fin.tile([P, G], f32, tag="diff")
            nc.vector.tensor_tensor(
                diff[:],
                se2_t[:].rearrange("p (g k) -> p g k", g=G)[:, :, T],
                lnS1[:], Alu.subtract)
            nc.vector.tensor_tensor(diff[:], diff[:], zpsA[:], Alu.add)
            dsum = fin.tile([P, 1], f32, tag="dsum")
            nc.vector.tensor_reduce(dsum[:], diff[:], AX, Alu.add)
            tot = fin.tile([P, 1], f32, tag="tot")
            nc.gpsimd.partition_all_reduce(tot[:], dsum[:], P,
                                           bass_isa.ReduceOp.add)
            nc.sync.dma_start(loss[:], tot[0:1, :])

            # ---- backtrace (arithmetic one-hot on DVE) ----
            h_t = st.tile([P, G * R], bf16, tag="h", name="h")
            nc.vector.tensor_copy(h_t[:], sbc["c_sigoh"][:])
            for t in range(L - 1, -1, -1):
                prod = scr.tile([P, G * R], bf16, tag="prod", name="prod")
                nc.vector.tensor_tensor(
                    prod[:], h_t[:], bp[:, t * G * R:(t + 1) * G * R],
                    Alu.mult)
                pcol = path[:, t * G:(t + 1) * G]
                nc.vector.tensor_reduce(
                    pcol, prod[:].rearrange("p (g k) -> p g k", g=G),
                    AX, Alu.add)
                if t > 0:
                    h_n = st.tile([P, G * R], bf16, tag="h", name="h")
                    nc.vector.tensor_tensor(
                        h_n[:].rearrange("p (g k) -> p g k", g=G),
                        sbc["c_idx"][:].rearrange("p (g k) -> p g k", g=G),
                        pcol.unsqueeze(2).broadcast_to((P, G, R)),
                        Alu.is_equal)
                    h_t = h_n

            # ---- finalize path: (path - g*11) * mask ----
            pv = path[:].rearrange("p (t g) -> p t g", g=G)
            nc.vector.tensor_tensor(
                pv, pv,
                sbc["c_g11"][:].unsqueeze(1).broadcast_to((P, L, G)),
                Alu.subtract)
            mskT = msk[:].rearrange("p (g t) -> p t g", g=G)
            nc.vector.tensor_tensor(pv, pv, mskT, Alu.mult)
            pathi = fin.tile([P, L * G], i32, tag="i32buf")
            nc.vector.tensor_copy(pathi[:], path[:])
            pathiv = pathi[:].rearrange("p (t g) -> p t g", g=G)
            for g in range(G):
                for h in range(2):
                    nc.sync.dma_start(
                        crf[g * P + h * 64:g * P + (h + 1) * 64, :],
                        pathiv[h * 64:(h + 1) * 64, :, g])
    nc.compile()
    return nc


def _get_prog():
    if "nc" not in _prog_cache:
        import concourse.bacc as bacc
        import concourse.mybir as mybir
        import concourse.tile as tile_mod
        nc = bacc.Bacc("TRN2", target_bir_lowering=False, debug=False,
                       enable_asserts=False)
        _build(nc, tile_mod, mybir)
        _prog_cache["nc"] = nc
    return _prog_cache["nc"]


def kernel(fc_out, tags, start_transitions, end_transitions, transitions):
    from concourse import bass_utils

    fc = np.ascontiguousarray(np.asarray(fc_out, dtype=np.float32))
    tg = np.ascontiguousarray(np.asarray(tags)).astype(np.int32)
    start = np.asarray(start_transitions, dtype=np.float32)
    end = np.asarray(end_transitions, dtype=np.float32)
    trans = np.asarray(transitions, dtype=np.float32)

    consts = _host_consts(start, end, trans)
    nc = _get_prog()

    in_maps = []
    for c in range(NCORE):
        m = {"fc": fc[c * SC:(c + 1) * SC],
             "tg": tg[c * SC:(c + 1) * SC]}
        m.update(consts)
        in_maps.append(m)

    res = bass_utils.run_bass_kernel_spmd(nc, in_maps,
                                          core_ids=list(range(NCORE)))
    outs = res.results
    crf_out = np.concatenate([outs[c]["crf"] for c in range(NCORE)], axis=0)
    lossv = -np.sum(np.stack([outs[c]["loss"].reshape(()) for c in
                              range(NCORE)]), dtype=np.float32)
    return (crf_out.astype(np.int32), np.float32(lossv))
